# revision 15
# baseline (speedup 1.0000x reference)
"""MultiHeadMlp TRN2 kernel: grouped per-head MLP + SE channel attention.

Full-input contract: kernel(**inputs) takes the complete arrays and returns
the complete output. Internally shards data-parallel over the batch dim
(B=8 -> 8 NeuronCores), builds one SPMD Bass/Tile program, and runs it via
run_bass_kernel_spmd.

Math (per batch element b, all tokens local to one core):
    xh = x.reshape(N, H, D)
    h  = gelu(xh @ W1 + b1)          per head, D=256 -> HID=1024
    o  = h @ W2 + b2                 per head, HID   -> D
    out = concat_heads(o)            (N, C)
    pooled = out.mean(axis=0)        (C,)
    gate = sigmoid(relu(pooled@cw1+cb1)@cw2+cb2)
    y = out * (1 + gate)

Layout strategy: everything on-chip is channel-major ("transposed"):
the host hands the kernel x^T (and un-transposes y^T on the way out), so
W1 [D,HID] / W2 [HID,D] serve directly as matmul lhsT operands, the SE
pool is a free-dim reduction, the gate is a native per-partition scalar
multiply, and the device never transposes anything.

Tail-overlap strategy: the SE gate depends on the token-mean of out, which
would serialize the entire 8 MB output DMA after the last GEMM2. Instead
GEMM2 for the last two token chunks is deferred until after the gate:
  - their GEMM1 runs spread through the main loop (2 extra heads per
    chunk) so the scalar engine's gelu stream never becomes the pacer,
    with h kept resident in SBUF and row-sums taken on the DVE;
  - pooled = (sum_chunk prow + hsum67 @ W2)/N + b2/4, where prow comes
    free from the pre-gate GEMM2 epilogues' accum_out and the matvec is
    64 N=1 matmuls interleaved into the last pre-gate chunk;
  - after the tiny SE MLP produces the gate, the deferred GEMM2 runs with
    (1+gate) folded into its ACT epilogue while the DVE rescales the six
    retained chunks - so the whole 8 MB output DMA overlaps the final
    ~27us of matmuls instead of trailing them.
"""

import numpy as np
import ml_dtypes

B = 8
N = 4096
DIM = 1024
H = 4
HD = 256           # head dim
HID = 1024         # per-head hidden
SQ = 64            # squeeze dim
TCH = 512          # tokens per chunk
NCHUNK = N // TCH  # 8
PRE = 6            # chunks whose GEMM2 runs before the gate
NCORES = 8

_BF = ml_dtypes.bfloat16

_cache = {}


def _build():
    from contextlib import ExitStack

    import concourse.bass as bass
    import concourse.mybir as mybir
    from concourse import bacc
    from concourse.tile import TileContext

    dt = mybir.dt
    bf = dt.bfloat16
    f32 = dt.float32
    Act = mybir.ActivationFunctionType
    Alu = mybir.AluOpType
    Ax = mybir.AxisListType

    nc = bacc.Bacc("TRN2", target_bir_lowering=False, debug=False)

    xt = nc.dram_tensor("xt", [DIM, N], bf, kind="ExternalInput")
    w1 = nc.dram_tensor("w1", [H, HD, HID], bf, kind="ExternalInput")
    w2 = nc.dram_tensor("w2", [H, HID, HD], bf, kind="ExternalInput")
    b1t = nc.dram_tensor("b1t", [128, H * 8], f32, kind="ExternalInput")
    b2t = nc.dram_tensor("b2t", [128, 8], f32, kind="ExternalInput")
    cw1 = nc.dram_tensor("cw1", [DIM, SQ], bf, kind="ExternalInput")
    cb1t = nc.dram_tensor("cb1t", [SQ, 1], f32, kind="ExternalInput")
    cw2 = nc.dram_tensor("cw2", [SQ, DIM], bf, kind="ExternalInput")
    cb2t = nc.dram_tensor("cb2t", [128, 8], f32, kind="ExternalInput")
    outT = nc.dram_tensor("outT", [DIM, N], bf, kind="ExternalOutput")

    with TileContext(nc) as tc, ExitStack() as ctx:
        const = ctx.enter_context(tc.tile_pool(name="const", bufs=1))
        xpool = ctx.enter_context(tc.tile_pool(name="xpool", bufs=2))
        hpool = ctx.enter_context(tc.tile_pool(name="hpool", bufs=2))
        pg1 = ctx.enter_context(tc.tile_pool(name="pg1", bufs=4, space="PSUM"))
        pg2 = ctx.enter_context(tc.tile_pool(name="pg2", bufs=3, space="PSUM"))
        pgp = ctx.enter_context(tc.tile_pool(name="pgp", bufs=1, space="PSUM"))

        # ---- activation-table + PE-clock warmup (overlaps the load phase) ----
        warm = const.tile([128, 1], f32, name="warm", tag="warm")
        nc.vector.memset(warm, 0.0)
        nc.scalar.activation(out=warm, in_=warm, func=Act.Sigmoid)
        nc.scalar.activation(out=warm, in_=warm, func=Act.Relu)
        nc.scalar.activation(out=warm, in_=warm, func=Act.Gelu)
        # dummy matmuls keep the PE busy through the HAM activity window so
        # the real GEMM stream starts at the warm 2.4 GHz clock
        wmm = const.tile([128, 512], bf, name="wmm", tag="wmm")
        nc.vector.memset(wmm, 0.0)
        for _ in range(4):
            pw = pg1.tile([128, 512], f32, name="p1", tag="p1")
            nc.tensor.matmul(pw, lhsT=wmm[:, 0:128], rhs=wmm,
                             start=True, stop=True)

        # ---- persistent tiles ----
        w1sb = [const.tile([128, 2, HID], bf, name=f"w1sb_{h}",
                           tag=f"w1sb_{h}") for h in range(H)]
        w2sb = [const.tile([128, 8, HD], bf, name=f"w2sb_{h}",
                           tag=f"w2sb_{h}") for h in range(H)]
        b1sb = const.tile([128, H * 8], f32, name="b1sb", tag="b1sb")
        b2sb = const.tile([128, 8], f32, name="b2sb", tag="b2sb")
        # channel-major out accumulator (pre-gate chunks only; the two
        # post-gate chunks stage through the by-then-dead x67 tiles)
        oT = [const.tile([128, PRE * TCH], bf, name=f"oT_{c}",
                         tag=f"oT_{c}") for c in range(8)]
        # x and h kept resident for the post-gate chunks
        x67 = {c: [const.tile([128, TCH], bf, name=f"x{c}_{ct}",
                              tag=f"x{c}_{ct}") for ct in range(8)]
               for c in (6, 7)}
        h67 = {c: [const.tile([128, TCH], bf, name=f"h{c}_{i}",
                              tag=f"h{c}_{i}") for i in range(32)]
               for c in (6, 7)}
        # per-(chunk, head, m) gelu row sums for chunks 6/7 (DVE-reduced)
        hrow = const.tile([128, 64], f32, name="hrow", tag="hrow")
        hsumb = const.tile([128, 32], bf, name="hsumb", tag="hsumb")
        # per-(chunk, ctile) out row sums from the pre-gate GEMM2 epilogues
        prow = const.tile([128, PRE * 8], f32, name="prow", tag="prow")
        prow3 = prow.rearrange("p (i q) -> p i q", q=8)
        prsum = const.tile([128, 8], f32, name="prsum", tag="prsum")
        cw1sb = const.tile([128, 8, SQ], bf, name="cw1sb", tag="cw1sb")
        cb1sb = const.tile([SQ, 1], f32, name="cb1sb", tag="cb1sb")
        cw2sb = const.tile([SQ, DIM], bf, name="cw2sb", tag="cw2sb")
        cb2sb = const.tile([128, 8], f32, name="cb2sb", tag="cb2sb")
        b2q = const.tile([128, 8], f32, name="b2q", tag="b2q")
        # pooled matvec accumulator (held across the interleaved emissions)
        poolp = pgp.tile([128, 8], f32, name="poolp", tag="poolp")

        def prefetch_x(c, eng=None):
            tiles = []
            for ct in range(8):
                if c >= PRE:
                    t = x67[c][ct]
                else:
                    t = xpool.tile([128, TCH], bf, name=f"x_{ct}",
                                   tag=f"x_{ct}")
                (eng or nc.sync).dma_start(
                    out=t, in_=xt[ct * 128:(ct + 1) * 128,
                                  c * TCH:(c + 1) * TCH])
                tiles.append(t)
            return tiles

        # ---- load order: one serial sync HWDGE ring, strict deadline
        # order (x chunk 0 first, then weights interleaved by first use) ----
        nc.sync.dma_start(out=b1sb, in_=b1t[:, :])
        w1r0 = w1[0].rearrange("(k p) n -> p k n", p=128)

        def x0_tile(ct):
            t = xpool.tile([128, TCH], bf, name=f"x_{ct}", tag=f"x_{ct}")
            nc.sync.dma_start(out=t, in_=xt[ct * 128:(ct + 1) * 128, 0:TCH])
            return t

        # head-0 weights split into m-pair slices interleaved with the
        # first x tiles: the first matmul only waits for ~400KB of DMA
        nc.sync.dma_start(out=w1sb[0][:, :, 0:256], in_=w1r0[:, :, 0:256])
        xcur = [None] * 8
        xcur[0] = x0_tile(0)
        xcur[1] = x0_tile(1)
        nc.sync.dma_start(out=b2sb, in_=b2t[:, :])
        for mp in range(1, 4):
            nc.sync.dma_start(out=w1sb[0][:, :, mp * 256:(mp + 1) * 256],
                              in_=w1r0[:, :, mp * 256:(mp + 1) * 256])
        w2r0 = w2[0].rearrange("(k p) n -> p k n", p=128)
        nc.sync.dma_start(out=w2sb[0][:, 0:4, :], in_=w2r0[:, 0:4, :])
        nc.sync.dma_start(out=w2sb[0][:, 4:8, :], in_=w2r0[:, 4:8, :])
        xcur[2] = x0_tile(2)
        xcur[3] = x0_tile(3)
        nc.sync.dma_start(out=w1sb[1],
                          in_=w1[1].rearrange("(k p) n -> p k n", p=128))
        xcur[4] = x0_tile(4)
        xcur[5] = x0_tile(5)
        nc.sync.dma_start(out=w2sb[1],
                          in_=w2[1].rearrange("(k p) n -> p k n", p=128))
        xcur[6] = x0_tile(6)
        xcur[7] = x0_tile(7)
        for h in range(2, H):
            nc.sync.dma_start(out=w1sb[h],
                              in_=w1[h].rearrange("(k p) n -> p k n", p=128))
            nc.sync.dma_start(out=w2sb[h],
                              in_=w2[h].rearrange("(k p) n -> p k n", p=128))
        xnext = prefetch_x(1)
        prefetch_x(6)
        nc.sync.dma_start(out=cb1sb, in_=cb1t[:, :])
        nc.sync.dma_start(out=cb2sb, in_=cb2t[:, :])
        nc.sync.dma_start(out=cw1sb,
                          in_=cw1.rearrange("(c p) n -> p c n", p=128))
        nc.sync.dma_start(out=cw2sb, in_=cw2[:, :])
        nc.vector.tensor_scalar_mul(b2q, b2sb, (NCHUNK - PRE) * TCH / N)

        def emit_g1(c, h, xts, dest):
            # GEMM1 head: dest[m] = gelu(W1_h^T x^T + b1) for 8 m-tiles
            for m in range(8):
                p1 = pg1.tile([128, TCH], f32, name="p1", tag="p1")
                nc.tensor.matmul(
                    p1, lhsT=w1sb[h][:, 0, m * 128:(m + 1) * 128],
                    rhs=xts[2 * h], start=True, stop=False)
                nc.tensor.matmul(
                    p1, lhsT=w1sb[h][:, 1, m * 128:(m + 1) * 128],
                    rhs=xts[2 * h + 1], start=False, stop=True)
                nc.scalar.activation(
                    out=dest[m], in_=p1, func=Act.Gelu,
                    bias=b1sb[:, h * 8 + m:h * 8 + m + 1])

        def emit_matvec(h):
            # pooled contribution of chunks 6/7: poolp[:, 2h+d] = sum_k
            # W2_h[k-block, d-half]^T hsum67_h[k-block]  (64 N=1 matmuls)
            for d in range(2):
                col = h * 2 + d
                for k in range(8):
                    nc.tensor.matmul(
                        poolp[:, col:col + 1],
                        lhsT=w2sb[h][:, k, d * 128:(d + 1) * 128],
                        rhs=hsumb[:, h * 8 + k:h * 8 + k + 1],
                        start=(k == 0), stop=(k == 7))

        # deferred-GEMM1 slots: 2 per chunk at c=1..4 covers chunks 6,7
        slots = {1: [(6, 0), (6, 1)], 2: [(6, 2), (6, 3)],
                 3: [(7, 0), (7, 1)], 4: [(7, 2), (7, 3)]}

        def emit_mix(cur, prev):
            # cur:  None | ("pre", c, h) | ("slot", dc, dh)  - a GEMM1 unit
            # prev: None | (c, h, ht)                        - a GEMM2 unit
            # Interleaving the GEMM1 matmul pairs with the previous unit's
            # GEMM2 k-chain halves the gelu-tile production rate so the
            # scalar engine (~590ns/tile) never stalls the PE.
            ht = None
            if cur is not None:
                if cur[0] == "pre":
                    _, c, h = cur
                    xts = xcur
                    dest = [hpool.tile([128, TCH], bf, name=f"ht_{m}",
                                       tag=f"ht_{m}") for m in range(8)]
                    ht = dest
                else:
                    _, dc, dh = cur
                    h = dh
                    xts = x67[dc]
                    dest = h67[dc][dh * 8:(dh + 1) * 8]
            if prev is not None:
                pc, ph, pht = prev
                p2 = [pg2.tile([128, TCH], f32, name="p2", tag="p2")
                      for _ in range(2)]
            for m in range(8):
                if cur is not None:
                    p1 = pg1.tile([128, TCH], f32, name="p1", tag="p1")
                    nc.tensor.matmul(
                        p1, lhsT=w1sb[h][:, 0, m * 128:(m + 1) * 128],
                        rhs=xts[2 * h], start=True, stop=False)
                    nc.tensor.matmul(
                        p1, lhsT=w1sb[h][:, 1, m * 128:(m + 1) * 128],
                        rhs=xts[2 * h + 1], start=False, stop=True)
                    nc.scalar.activation(
                        out=dest[m], in_=p1, func=Act.Gelu,
                        bias=b1sb[:, h * 8 + m:h * 8 + m + 1])
                if prev is not None:
                    for d in range(2):
                        nc.tensor.matmul(
                            p2[d],
                            lhsT=w2sb[ph][:, m, d * 128:(d + 1) * 128],
                            rhs=pht[m], start=(m == 0), stop=(m == 7))
            if prev is not None:
                for d in range(2):
                    ctile = ph * 2 + d
                    idx = pc * 8 + ctile
                    nc.vector.tensor_scalar(
                        out=oT[ctile][:, pc * TCH:(pc + 1) * TCH],
                        in0=p2[d], scalar1=b2sb[:, ctile:ctile + 1],
                        scalar2=0.0, op0=Alu.add, op1=Alu.add,
                        accum_out=prow[:, idx:idx + 1])
            if cur is not None and cur[0] == "slot":
                for m in range(8):
                    col = (dc - 6) * 32 + dh * 8 + m
                    nc.vector.tensor_reduce(
                        out=hrow[:, col:col + 1], in_=dest[m],
                        axis=Ax.X, op=Alu.add)
            return ht

        # ---- main loop: GEMM1 units in order, each interleaved with the
        # oldest pending GEMM2 unit ----
        g2q = []
        for c in range(PRE):
            if 1 <= c < PRE - 1:
                xnext = prefetch_x(c + 1)
            if c == 2:
                prefetch_x(7)
            sl = slots.get(c, ())
            units = [("pre", c, 0), ("pre", c, 1)]
            if sl:
                units.append(("slot",) + sl[0])
            units += [("pre", c, 2), ("pre", c, 3)]
            if sl:
                units.append(("slot",) + sl[1])
            for u in units:
                prev = g2q.pop(0) if g2q else None
                ht = emit_mix(u, prev)
                if u[0] == "pre":
                    g2q.append((u[1], u[2], ht))
            if c == PRE - 2:
                # chunks 6+7 h sums, bf16, ready before the matvec emissions
                for h in range(H):
                    nc.vector.tensor_tensor(
                        out=hsumb[:, h * 8:(h + 1) * 8],
                        in0=hrow[:, h * 8:h * 8 + 8],
                        in1=hrow[:, 32 + h * 8:32 + h * 8 + 8], op=Alu.add)
            xcur = xnext

        # drain: the last pending GEMM2 interleaved with the pooled matvec
        assert len(g2q) == 1
        pc, ph, pht = g2q[0]
        p2 = [pg2.tile([128, TCH], f32, name="p2", tag="p2")
              for _ in range(2)]
        mvlist = [(h, d, k) for h in range(H) for d in range(2)
                  for k in range(8)]
        for m in range(8):
            for d in range(2):
                nc.tensor.matmul(
                    p2[d], lhsT=w2sb[ph][:, m, d * 128:(d + 1) * 128],
                    rhs=pht[m], start=(m == 0), stop=(m == 7))
            for (h, d, k) in mvlist[m * 8:(m + 1) * 8]:
                nc.tensor.matmul(
                    poolp[:, h * 2 + d:h * 2 + d + 1],
                    lhsT=w2sb[h][:, k, d * 128:(d + 1) * 128],
                    rhs=hsumb[:, h * 8 + k:h * 8 + k + 1],
                    start=(k == 0), stop=(k == 7))
        for d in range(2):
            ctile = ph * 2 + d
            idx = pc * 8 + ctile
            nc.vector.tensor_scalar(
                out=oT[ctile][:, pc * TCH:(pc + 1) * TCH], in0=p2[d],
                scalar1=b2sb[:, ctile:ctile + 1],
                scalar2=0.0, op0=Alu.add, op1=Alu.add,
                accum_out=prow[:, idx:idx + 1])

        # prefetch the sigmoid table set (the gelu set is resident; without
        # this the 1.3us table load lands between relu and the gate sigmoid)
        nc.scalar.activation(out=warm, in_=warm, func=Act.Sigmoid)

        # ---- SE channel attention:
        # pooled = (sum_c prow_c + poolp)/N + b2/4 ----
        for q in range(8):
            nc.vector.tensor_reduce(out=prsum[:, q:q + 1],
                                    in_=prow3[:, 0:PRE, q],
                                    axis=Ax.X, op=Alu.add)
        pool1 = const.tile([128, 8], f32, name="pool1", tag="pool1")
        nc.vector.tensor_tensor(out=pool1, in0=prsum, in1=poolp, op=Alu.add)
        pooltmp = const.tile([128, 8], f32, name="pooltmp", tag="pooltmp")
        nc.vector.tensor_scalar_mul(pooltmp, pool1, 1.0 / N)
        pooledT = const.tile([128, 8], bf, name="pooledT", tag="pooledT")
        nc.vector.tensor_tensor(out=pooledT, in0=pooltmp, in1=b2q,
                                op=Alu.add)

        pz = pg1.tile([SQ, 1], f32, name="pz", tag="p1")
        for cb in range(8):
            nc.tensor.matmul(pz, lhsT=cw1sb[:, cb, :],
                             rhs=pooledT[:, cb:cb + 1],
                             start=(cb == 0), stop=(cb == 7))
        z1sb = const.tile([SQ, 1], bf, name="z1sb", tag="z1sb")
        nc.scalar.activation(out=z1sb, in_=pz, func=Act.Relu, bias=cb1sb)

        gp8 = pgp.tile([128, 8], f32, name="gp8", tag="poolp")
        for cb in range(8):
            nc.tensor.matmul(gp8[:, cb:cb + 1],
                             lhsT=cw2sb[:, cb * 128:(cb + 1) * 128],
                             rhs=z1sb, start=True, stop=True)
        gadd = const.tile([128, 8], f32, name="gadd", tag="gadd")
        nc.vector.tensor_tensor(out=gadd, in0=gp8, in1=cb2sb, op=Alu.add)
        g1T = const.tile([128, 8], f32, name="g1T", tag="g1T")
        nc.scalar.activation(out=g1T, in_=gadd, func=Act.Sigmoid)
        nc.vector.tensor_scalar_add(g1T, g1T, 1.0)
        # gated bias: b2*(1+g), so (p2 + b2)*(1+g) = p2*g1 + b2g
        b2g = const.tile([128, 8], f32, name="b2g", tag="b2g")
        nc.vector.tensor_tensor(out=b2g, in0=b2sb, in1=g1T, op=Alu.mult)

        # ---- post-gate: GEMM2 for chunks 6,7 (gate folded into the ACT
        # epilogue) while the DVE rescales chunks 0..5 in place; every piece
        # is DMA'd out as soon as it is gated, overlapping the matmuls ----
        PW = (PRE * TCH) // 2  # in-place scale piece width
        for pct in range(8):
            for ph in range(2):
                psl = slice(ph * PW, (ph + 1) * PW)
                nc.vector.tensor_scalar(
                    out=oT[pct][:, psl], in0=oT[pct][:, psl],
                    scalar1=g1T[:, pct:pct + 1], scalar2=0.0,
                    op0=Alu.mult, op1=Alu.add)
                nc.sync.dma_start(
                    out=outT[pct * 128:(pct + 1) * 128, psl],
                    in_=oT[pct][:, psl])
        for c in (6, 7):
            for h in range(H):
                for d in range(2):
                    ctile = h * 2 + d
                    p2 = pg2.tile([128, TCH], f32, name="p2", tag="p2")
                    for k in range(8):
                        nc.tensor.matmul(
                            p2, lhsT=w2sb[h][:, k, d * 128:(d + 1) * 128],
                            rhs=h67[c][h * 8 + k],
                            start=(k == 0), stop=(k == 7))
                    st = x67[c][ctile]
                    nc.scalar.activation(
                        out=st, in_=p2, func=Act.Identity,
                        scale=g1T[:, ctile:ctile + 1],
                        bias=b2g[:, ctile:ctile + 1])
                    nc.scalar.dma_start(
                        out=outT[ctile * 128:(ctile + 1) * 128,
                                 c * TCH:(c + 1) * TCH],
                        in_=st)

    nc.compile()
    return nc


def _get_nc():
    if "nc" not in _cache:
        _cache["nc"] = _build()
    return _cache["nc"]


def _make_in_maps(x, W1, b1, W2, b2, cw1, cb1, cw2, cb2):
    # bf16 + pre-transposed x: (B, N, DIM) -> per-core (DIM, N)
    xb = np.asarray(x, dtype=_BF)
    w1b = np.asarray(W1, dtype=_BF)
    w2b = np.asarray(W2, dtype=_BF)
    cw1b = np.asarray(cw1, dtype=_BF)
    cw2b = np.asarray(cw2, dtype=_BF)
    b1tv = np.ascontiguousarray(
        np.asarray(b1, np.float32).reshape(H, 8, 128).transpose(2, 0, 1)
        .reshape(128, H * 8))
    b2tv = np.ascontiguousarray(
        np.asarray(b2, np.float32).reshape(H, 2, 128).transpose(2, 0, 1)
        .reshape(128, 8))
    cb1v = np.asarray(cb1, np.float32).reshape(SQ, 1)
    cb2tv = np.ascontiguousarray(
        np.asarray(cb2, np.float32).reshape(8, 128).T)

    shared = {
        "w1": w1b, "w2": w2b, "b1t": b1tv, "b2t": b2tv,
        "cw1": cw1b, "cb1t": cb1v, "cw2": cw2b, "cb2t": cb2tv,
    }
    return [dict(shared, xt=np.ascontiguousarray(xb[i].T))
            for i in range(NCORES)]


def kernel(x, W1, b1, W2, b2, cw1, cb1, cw2, cb2):
    from concourse.bass_utils import run_bass_kernel_spmd

    nc = _get_nc()
    in_maps = _make_in_maps(x, W1, b1, W2, b2, cw1, cb1, cw2, cb2)
    res = run_bass_kernel_spmd(nc, in_maps, core_ids=list(range(NCORES)))
    # un-transpose: per-core (DIM, N) -> (N, DIM)
    y = np.stack([res.results[i]["outT"].T for i in range(NCORES)], axis=0)
    return y.astype(np.float32)


# revision 16
# speedup vs baseline: 1.1841x; 1.1841x over previous
"""MultiHeadMlp TRN2 kernel: grouped per-head MLP + SE channel attention.

Full-input contract: kernel(**inputs) takes the complete arrays and returns
the complete output. Internally shards data-parallel over the batch dim
(B=8 -> 8 NeuronCores), builds one SPMD Bass/Tile program, and runs it via
run_bass_kernel_spmd.

Math (per batch element b, all tokens local to one core):
    xh = x.reshape(N, H, D)
    h  = gelu(xh @ W1 + b1)          per head, D=256 -> HID=1024
    o  = h @ W2 + b2                 per head, HID   -> D
    out = concat_heads(o)            (N, C)
    pooled = out.mean(axis=0)        (C,)
    gate = sigmoid(relu(pooled@cw1+cb1)@cw2+cb2)
    y = out * (1 + gate)

Layout strategy: everything on-chip is channel-major ("transposed"):
the host hands the kernel x^T (and un-transposes y^T on the way out), so
W1 [D,HID] / W2 [HID,D] serve directly as matmul lhsT operands, the SE
pool is a free-dim reduction, the gate is a native per-partition scalar
multiply, and the device never transposes anything.

Tail-overlap strategy: the SE gate depends on the token-mean of out, which
would serialize the entire 8 MB output DMA after the last GEMM2. Instead
GEMM2 for the last two token chunks is deferred until after the gate:
  - their GEMM1 runs spread through the main loop (2 extra heads per
    chunk) so the scalar engine's gelu stream never becomes the pacer,
    with h kept resident in SBUF and row-sums taken on the DVE;
  - pooled = (sum_chunk prow + hsum67 @ W2)/N + b2/4, where prow comes
    free from the pre-gate GEMM2 epilogues' accum_out and the matvec is
    64 N=1 matmuls interleaved into the last pre-gate chunk;
  - after the tiny SE MLP produces the gate, the deferred GEMM2 runs with
    (1+gate) folded into its ACT epilogue while the DVE rescales the six
    retained chunks - so the whole 8 MB output DMA overlaps the final
    ~27us of matmuls instead of trailing them.
"""

import numpy as np
import ml_dtypes

B = 8
N = 4096
DIM = 1024
H = 4
HD = 256           # head dim
HID = 1024         # per-head hidden
SQ = 64            # squeeze dim
TCH = 512          # tokens per chunk
NCHUNK = N // TCH  # 8
PRE = 6            # chunks whose GEMM2 runs before the gate
NCORES = 8

_BF = ml_dtypes.bfloat16

_cache = {}


def _build():
    from contextlib import ExitStack

    import concourse.bass as bass
    import concourse.mybir as mybir
    from concourse import bacc
    from concourse.tile import TileContext

    dt = mybir.dt
    bf = dt.bfloat16
    f32 = dt.float32
    Act = mybir.ActivationFunctionType
    Alu = mybir.AluOpType
    Ax = mybir.AxisListType

    nc = bacc.Bacc("TRN2", target_bir_lowering=False, debug=False)

    xt = nc.dram_tensor("xt", [DIM, N], bf, kind="ExternalInput")
    w1 = nc.dram_tensor("w1", [H, HD, HID], bf, kind="ExternalInput")
    w2 = nc.dram_tensor("w2", [H, HID, HD], bf, kind="ExternalInput")
    b1t = nc.dram_tensor("b1t", [128, H * 8], f32, kind="ExternalInput")
    b2t = nc.dram_tensor("b2t", [128, 8], f32, kind="ExternalInput")
    cw1 = nc.dram_tensor("cw1", [DIM, SQ], bf, kind="ExternalInput")
    cb1t = nc.dram_tensor("cb1t", [SQ, 1], f32, kind="ExternalInput")
    cw2 = nc.dram_tensor("cw2", [SQ, DIM], bf, kind="ExternalInput")
    cb2t = nc.dram_tensor("cb2t", [128, 8], f32, kind="ExternalInput")
    outT = nc.dram_tensor("outT", [DIM, N], bf, kind="ExternalOutput")

    with TileContext(nc) as tc, ExitStack() as ctx:
        const = ctx.enter_context(tc.tile_pool(name="const", bufs=1))
        xpool = ctx.enter_context(tc.tile_pool(name="xpool", bufs=2))
        hpool = ctx.enter_context(tc.tile_pool(name="hpool", bufs=2))
        pg1 = ctx.enter_context(tc.tile_pool(name="pg1", bufs=4, space="PSUM"))
        pg2 = ctx.enter_context(tc.tile_pool(name="pg2", bufs=3, space="PSUM"))
        pgp = ctx.enter_context(tc.tile_pool(name="pgp", bufs=1, space="PSUM"))

        # ---- activation-table + PE-clock warmup (overlaps the load phase) ----
        warm = const.tile([128, 1], f32, name="warm", tag="warm")
        nc.vector.memset(warm, 0.0)
        nc.scalar.activation(out=warm, in_=warm, func=Act.Sigmoid)
        nc.scalar.activation(out=warm, in_=warm, func=Act.Relu)
        nc.scalar.activation(out=warm, in_=warm, func=Act.Gelu)
        # dummy matmuls keep the PE busy through the HAM activity window so
        # the real GEMM stream starts at the warm 2.4 GHz clock
        wmm = const.tile([128, 512], bf, name="wmm", tag="wmm")
        nc.vector.memset(wmm, 0.0)
        for _ in range(6):
            pw = pg1.tile([128, 512], f32, name="p1", tag="p1")
            nc.tensor.matmul(pw, lhsT=wmm[:, 0:128], rhs=wmm,
                             start=True, stop=True)

        # ---- persistent tiles ----
        w1sb = [const.tile([128, 2, HID], bf, name=f"w1sb_{h}",
                           tag=f"w1sb_{h}") for h in range(H)]
        w2sb = [const.tile([128, 8, HD], bf, name=f"w2sb_{h}",
                           tag=f"w2sb_{h}") for h in range(H)]
        b1sb = const.tile([128, H * 8], f32, name="b1sb", tag="b1sb")
        b2sb = const.tile([128, 8], f32, name="b2sb", tag="b2sb")
        # channel-major out accumulator (pre-gate chunks only; the two
        # post-gate chunks stage through the by-then-dead x67 tiles)
        oT = [const.tile([128, PRE * TCH], bf, name=f"oT_{c}",
                         tag=f"oT_{c}") for c in range(8)]
        # x and h kept resident for the post-gate chunks
        x67 = {c: [const.tile([128, TCH], bf, name=f"x{c}_{ct}",
                              tag=f"x{c}_{ct}") for ct in range(8)]
               for c in (6, 7)}
        h67 = {c: [const.tile([128, TCH], bf, name=f"h{c}_{i}",
                              tag=f"h{c}_{i}") for i in range(32)]
               for c in (6, 7)}
        # per-(chunk, head, m) gelu row sums for chunks 6/7 (DVE-reduced)
        hrow = const.tile([128, 64], f32, name="hrow", tag="hrow")
        hsumb = const.tile([128, 32], bf, name="hsumb", tag="hsumb")
        # per-(chunk, ctile) out row sums from the pre-gate GEMM2 epilogues
        prow = const.tile([128, PRE * 8], f32, name="prow", tag="prow")
        prow3 = prow.rearrange("p (i q) -> p i q", q=8)
        prsum = const.tile([128, 8], f32, name="prsum", tag="prsum")
        cw1sb = const.tile([128, 8, SQ], bf, name="cw1sb", tag="cw1sb")
        cb1sb = const.tile([SQ, 1], f32, name="cb1sb", tag="cb1sb")
        cw2sb = const.tile([SQ, DIM], bf, name="cw2sb", tag="cw2sb")
        cb2sb = const.tile([128, 8], f32, name="cb2sb", tag="cb2sb")
        b2q = const.tile([128, 8], f32, name="b2q", tag="b2q")
        # pooled matvec accumulator (held across the interleaved emissions)
        poolp = pgp.tile([128, 8], f32, name="poolp", tag="poolp")

        def prefetch_x(c, eng=None):
            tiles = []
            for ct in range(8):
                if c >= PRE:
                    t = x67[c][ct]
                else:
                    t = xpool.tile([128, TCH], bf, name=f"x_{ct}",
                                   tag=f"x_{ct}")
                (eng or nc.sync).dma_start(
                    out=t, in_=xt[ct * 128:(ct + 1) * 128,
                                  c * TCH:(c + 1) * TCH])
                tiles.append(t)
            return tiles

        # ---- load order: one serial sync HWDGE ring, strict deadline
        # order (x chunk 0 first, then weights interleaved by first use) ----
        nc.sync.dma_start(out=b1sb, in_=b1t[:, :])
        w1r0 = w1[0].rearrange("(k p) n -> p k n", p=128)
        nc.sync.dma_start(out=w1sb[0][:, 0:1, :], in_=w1r0[:, 0:1, :])
        nc.sync.dma_start(out=w1sb[0][:, 1:2, :], in_=w1r0[:, 1:2, :])
        nc.sync.dma_start(out=b2sb, in_=b2t[:, :])

        def x0_tile(ct):
            t = xpool.tile([128, TCH], bf, name=f"x_{ct}", tag=f"x_{ct}")
            nc.sync.dma_start(out=t, in_=xt[ct * 128:(ct + 1) * 128, 0:TCH])
            return t

        xcur = [None] * 8
        xcur[0] = x0_tile(0)
        xcur[1] = x0_tile(1)
        w2r0 = w2[0].rearrange("(k p) n -> p k n", p=128)
        nc.sync.dma_start(out=w2sb[0][:, 0:4, :], in_=w2r0[:, 0:4, :])
        nc.sync.dma_start(out=w2sb[0][:, 4:8, :], in_=w2r0[:, 4:8, :])
        xcur[2] = x0_tile(2)
        xcur[3] = x0_tile(3)
        nc.sync.dma_start(out=w1sb[1],
                          in_=w1[1].rearrange("(k p) n -> p k n", p=128))
        xcur[4] = x0_tile(4)
        xcur[5] = x0_tile(5)
        nc.sync.dma_start(out=w2sb[1],
                          in_=w2[1].rearrange("(k p) n -> p k n", p=128))
        xcur[6] = x0_tile(6)
        xcur[7] = x0_tile(7)
        for h in range(2, H):
            nc.sync.dma_start(out=w1sb[h],
                              in_=w1[h].rearrange("(k p) n -> p k n", p=128))
            nc.sync.dma_start(out=w2sb[h],
                              in_=w2[h].rearrange("(k p) n -> p k n", p=128))
        xnext = prefetch_x(1)
        prefetch_x(6)
        nc.sync.dma_start(out=cb1sb, in_=cb1t[:, :])
        nc.sync.dma_start(out=cb2sb, in_=cb2t[:, :])
        nc.sync.dma_start(out=cw1sb,
                          in_=cw1.rearrange("(c p) n -> p c n", p=128))
        nc.sync.dma_start(out=cw2sb, in_=cw2[:, :])
        nc.vector.tensor_scalar_mul(b2q, b2sb, (NCHUNK - PRE) * TCH / N)

        def emit_g1(c, h, xts, dest):
            # GEMM1 head: dest[m] = gelu(W1_h^T x^T + b1) for 8 m-tiles
            for m in range(8):
                p1 = pg1.tile([128, TCH], f32, name="p1", tag="p1")
                nc.tensor.matmul(
                    p1, lhsT=w1sb[h][:, 0, m * 128:(m + 1) * 128],
                    rhs=xts[2 * h], start=True, stop=False)
                nc.tensor.matmul(
                    p1, lhsT=w1sb[h][:, 1, m * 128:(m + 1) * 128],
                    rhs=xts[2 * h + 1], start=False, stop=True)
                nc.scalar.activation(
                    out=dest[m], in_=p1, func=Act.Gelu,
                    bias=b1sb[:, h * 8 + m:h * 8 + m + 1])

        def emit_matvec(h):
            # pooled contribution of chunks 6/7: poolp[:, 2h+d] = sum_k
            # W2_h[k-block, d-half]^T hsum67_h[k-block]  (64 N=1 matmuls)
            for d in range(2):
                col = h * 2 + d
                for k in range(8):
                    nc.tensor.matmul(
                        poolp[:, col:col + 1],
                        lhsT=w2sb[h][:, k, d * 128:(d + 1) * 128],
                        rhs=hsumb[:, h * 8 + k:h * 8 + k + 1],
                        start=(k == 0), stop=(k == 7))

        # deferred-GEMM1 slots: 2 per chunk at c=1..4 covers chunks 6,7
        slots = {1: [(6, 0), (6, 1)], 2: [(6, 2), (6, 3)],
                 3: [(7, 0), (7, 1)], 4: [(7, 2), (7, 3)]}

        def emit_mix(cur, prev):
            # cur:  None | ("pre", c, h) | ("slot", dc, dh)  - a GEMM1 unit
            # prev: None | (c, h, ht)                        - a GEMM2 unit
            # Interleaving the GEMM1 matmul pairs with the previous unit's
            # GEMM2 k-chain halves the gelu-tile production rate so the
            # scalar engine (~590ns/tile) never stalls the PE.
            ht = None
            if cur is not None:
                if cur[0] == "pre":
                    _, c, h = cur
                    xts = xcur
                    dest = [hpool.tile([128, TCH], bf, name=f"ht_{m}",
                                       tag=f"ht_{m}") for m in range(8)]
                    ht = dest
                else:
                    _, dc, dh = cur
                    h = dh
                    xts = x67[dc]
                    dest = h67[dc][dh * 8:(dh + 1) * 8]
            if prev is not None:
                pc, ph, pht = prev
                p2 = [pg2.tile([128, TCH], f32, name="p2", tag="p2")
                      for _ in range(2)]
            for m in range(8):
                if cur is not None:
                    p1 = pg1.tile([128, TCH], f32, name="p1", tag="p1")
                    nc.tensor.matmul(
                        p1, lhsT=w1sb[h][:, 0, m * 128:(m + 1) * 128],
                        rhs=xts[2 * h], start=True, stop=False)
                    nc.tensor.matmul(
                        p1, lhsT=w1sb[h][:, 1, m * 128:(m + 1) * 128],
                        rhs=xts[2 * h + 1], start=False, stop=True)
                    nc.scalar.activation(
                        out=dest[m], in_=p1, func=Act.Gelu,
                        bias=b1sb[:, h * 8 + m:h * 8 + m + 1])
                if prev is not None:
                    for d in range(2):
                        nc.tensor.matmul(
                            p2[d],
                            lhsT=w2sb[ph][:, m, d * 128:(d + 1) * 128],
                            rhs=pht[m], start=(m == 0), stop=(m == 7))
            if prev is not None:
                for d in range(2):
                    ctile = ph * 2 + d
                    idx = pc * 8 + ctile
                    nc.vector.tensor_scalar(
                        out=oT[ctile][:, pc * TCH:(pc + 1) * TCH],
                        in0=p2[d], scalar1=b2sb[:, ctile:ctile + 1],
                        scalar2=0.0, op0=Alu.add, op1=Alu.add,
                        accum_out=prow[:, idx:idx + 1])
            if cur is not None and cur[0] == "slot":
                for m in range(8):
                    col = (dc - 6) * 32 + dh * 8 + m
                    nc.vector.tensor_reduce(
                        out=hrow[:, col:col + 1], in_=dest[m],
                        axis=Ax.X, op=Alu.add)
            return ht

        # ---- main loop: GEMM1 units in order, each interleaved with the
        # oldest pending GEMM2 unit ----
        g2q = []
        for c in range(PRE):
            if 1 <= c < PRE - 1:
                xnext = prefetch_x(c + 1)
            if c == 2:
                prefetch_x(7)
            sl = slots.get(c, ())
            units = [("pre", c, 0), ("pre", c, 1)]
            if sl:
                units.append(("slot",) + sl[0])
            units += [("pre", c, 2), ("pre", c, 3)]
            if sl:
                units.append(("slot",) + sl[1])
            for u in units:
                prev = g2q.pop(0) if g2q else None
                ht = emit_mix(u, prev)
                if u[0] == "pre":
                    g2q.append((u[1], u[2], ht))
            if c == PRE - 2:
                # chunks 6+7 h sums, bf16, ready before the matvec emissions
                for h in range(H):
                    nc.vector.tensor_tensor(
                        out=hsumb[:, h * 8:(h + 1) * 8],
                        in0=hrow[:, h * 8:h * 8 + 8],
                        in1=hrow[:, 32 + h * 8:32 + h * 8 + 8], op=Alu.add)
            xcur = xnext

        # drain: the last pending GEMM2 interleaved with the pooled matvec
        assert len(g2q) == 1
        pc, ph, pht = g2q[0]
        p2 = [pg2.tile([128, TCH], f32, name="p2", tag="p2")
              for _ in range(2)]
        mvlist = [(h, d, k) for h in range(H) for d in range(2)
                  for k in range(8)]
        for m in range(8):
            for d in range(2):
                nc.tensor.matmul(
                    p2[d], lhsT=w2sb[ph][:, m, d * 128:(d + 1) * 128],
                    rhs=pht[m], start=(m == 0), stop=(m == 7))
            for (h, d, k) in mvlist[m * 8:(m + 1) * 8]:
                nc.tensor.matmul(
                    poolp[:, h * 2 + d:h * 2 + d + 1],
                    lhsT=w2sb[h][:, k, d * 128:(d + 1) * 128],
                    rhs=hsumb[:, h * 8 + k:h * 8 + k + 1],
                    start=(k == 0), stop=(k == 7))
        for d in range(2):
            ctile = ph * 2 + d
            idx = pc * 8 + ctile
            nc.vector.tensor_scalar(
                out=oT[ctile][:, pc * TCH:(pc + 1) * TCH], in0=p2[d],
                scalar1=b2sb[:, ctile:ctile + 1],
                scalar2=0.0, op0=Alu.add, op1=Alu.add,
                accum_out=prow[:, idx:idx + 1])

        # prefetch the sigmoid table set (the gelu set is resident; without
        # this the 1.3us table load lands between relu and the gate sigmoid)
        nc.scalar.activation(out=warm, in_=warm, func=Act.Sigmoid)

        # ---- SE channel attention:
        # pooled = (sum_c prow_c + poolp)/N + b2/4 ----
        for q in range(8):
            nc.vector.tensor_reduce(out=prsum[:, q:q + 1],
                                    in_=prow3[:, 0:PRE, q],
                                    axis=Ax.X, op=Alu.add)
        pool1 = const.tile([128, 8], f32, name="pool1", tag="pool1")
        nc.vector.tensor_tensor(out=pool1, in0=prsum, in1=poolp, op=Alu.add)
        pooltmp = const.tile([128, 8], f32, name="pooltmp", tag="pooltmp")
        nc.vector.tensor_scalar_mul(pooltmp, pool1, 1.0 / N)
        pooledT = const.tile([128, 8], bf, name="pooledT", tag="pooledT")
        nc.vector.tensor_tensor(out=pooledT, in0=pooltmp, in1=b2q,
                                op=Alu.add)

        pz = pg1.tile([SQ, 1], f32, name="pz", tag="p1")
        for cb in range(8):
            nc.tensor.matmul(pz, lhsT=cw1sb[:, cb, :],
                             rhs=pooledT[:, cb:cb + 1],
                             start=(cb == 0), stop=(cb == 7))
        z1sb = const.tile([SQ, 1], bf, name="z1sb", tag="z1sb")
        nc.scalar.activation(out=z1sb, in_=pz, func=Act.Relu, bias=cb1sb)

        gp8 = pgp.tile([128, 8], f32, name="gp8", tag="poolp")
        for cb in range(8):
            nc.tensor.matmul(gp8[:, cb:cb + 1],
                             lhsT=cw2sb[:, cb * 128:(cb + 1) * 128],
                             rhs=z1sb, start=True, stop=True)
        gadd = const.tile([128, 8], f32, name="gadd", tag="gadd")
        nc.vector.tensor_tensor(out=gadd, in0=gp8, in1=cb2sb, op=Alu.add)
        g1T = const.tile([128, 8], f32, name="g1T", tag="g1T")
        nc.scalar.activation(out=g1T, in_=gadd, func=Act.Sigmoid)
        nc.vector.tensor_scalar_add(g1T, g1T, 1.0)
        # gated bias: b2*(1+g), so (p2 + b2)*(1+g) = p2*g1 + b2g
        b2g = const.tile([128, 8], f32, name="b2g", tag="b2g")
        nc.vector.tensor_tensor(out=b2g, in0=b2sb, in1=g1T, op=Alu.mult)

        # ---- post-gate: GEMM2 for chunks 6,7 (gate folded into the ACT
        # epilogue) while the DVE rescales chunks 0..5 in place; every piece
        # is DMA'd out as soon as it is gated, overlapping the matmuls ----
        PW = (PRE * TCH) // 2  # in-place scale piece width
        for pct in range(8):
            for ph in range(2):
                psl = slice(ph * PW, (ph + 1) * PW)
                nc.vector.tensor_scalar(
                    out=oT[pct][:, psl], in0=oT[pct][:, psl],
                    scalar1=g1T[:, pct:pct + 1], scalar2=0.0,
                    op0=Alu.mult, op1=Alu.add)
                nc.sync.dma_start(
                    out=outT[pct * 128:(pct + 1) * 128, psl],
                    in_=oT[pct][:, psl])
        for c in (6, 7):
            for h in range(H):
                for d in range(2):
                    ctile = h * 2 + d
                    p2 = pg2.tile([128, TCH], f32, name="p2", tag="p2")
                    for k in range(8):
                        nc.tensor.matmul(
                            p2, lhsT=w2sb[h][:, k, d * 128:(d + 1) * 128],
                            rhs=h67[c][h * 8 + k],
                            start=(k == 0), stop=(k == 7))
                    st = x67[c][ctile]
                    nc.scalar.activation(
                        out=st, in_=p2, func=Act.Identity,
                        scale=g1T[:, ctile:ctile + 1],
                        bias=b2g[:, ctile:ctile + 1])
                    nc.scalar.dma_start(
                        out=outT[ctile * 128:(ctile + 1) * 128,
                                 c * TCH:(c + 1) * TCH],
                        in_=st)

    nc.compile()
    return nc


def _get_nc():
    if "nc" not in _cache:
        _cache["nc"] = _build()
    return _cache["nc"]


def _make_in_maps(x, W1, b1, W2, b2, cw1, cb1, cw2, cb2):
    # bf16 + pre-transposed x: (B, N, DIM) -> per-core (DIM, N)
    xb = np.asarray(x, dtype=_BF)
    w1b = np.asarray(W1, dtype=_BF)
    w2b = np.asarray(W2, dtype=_BF)
    cw1b = np.asarray(cw1, dtype=_BF)
    cw2b = np.asarray(cw2, dtype=_BF)
    b1tv = np.ascontiguousarray(
        np.asarray(b1, np.float32).reshape(H, 8, 128).transpose(2, 0, 1)
        .reshape(128, H * 8))
    b2tv = np.ascontiguousarray(
        np.asarray(b2, np.float32).reshape(H, 2, 128).transpose(2, 0, 1)
        .reshape(128, 8))
    cb1v = np.asarray(cb1, np.float32).reshape(SQ, 1)
    cb2tv = np.ascontiguousarray(
        np.asarray(cb2, np.float32).reshape(8, 128).T)

    shared = {
        "w1": w1b, "w2": w2b, "b1t": b1tv, "b2t": b2tv,
        "cw1": cw1b, "cb1t": cb1v, "cw2": cw2b, "cb2t": cb2tv,
    }
    return [dict(shared, xt=np.ascontiguousarray(xb[i].T))
            for i in range(NCORES)]


def kernel(x, W1, b1, W2, b2, cw1, cb1, cw2, cb2):
    from concourse.bass_utils import run_bass_kernel_spmd

    nc = _get_nc()
    in_maps = _make_in_maps(x, W1, b1, W2, b2, cw1, cb1, cw2, cb2)
    res = run_bass_kernel_spmd(nc, in_maps, core_ids=list(range(NCORES)))
    # un-transpose: per-core (DIM, N) -> (N, DIM)
    y = np.stack([res.results[i]["outT"].T for i in range(NCORES)], axis=0)
    return y.astype(np.float32)


# revision 17
# speedup vs baseline: 1.1919x; 1.0066x over previous
"""MultiHeadMlp TRN2 kernel: grouped per-head MLP + SE channel attention.

Full-input contract: kernel(**inputs) takes the complete arrays and returns
the complete output. Internally shards data-parallel over the batch dim
(B=8 -> 8 NeuronCores), builds one SPMD Bass/Tile program, and runs it via
run_bass_kernel_spmd.

Math (per batch element b, all tokens local to one core):
    xh = x.reshape(N, H, D)
    h  = gelu(xh @ W1 + b1)          per head, D=256 -> HID=1024
    o  = h @ W2 + b2                 per head, HID   -> D
    out = concat_heads(o)            (N, C)
    pooled = out.mean(axis=0)        (C,)
    gate = sigmoid(relu(pooled@cw1+cb1)@cw2+cb2)
    y = out * (1 + gate)

Layout strategy: everything on-chip is channel-major ("transposed"):
the host hands the kernel x^T (and un-transposes y^T on the way out), so
W1 [D,HID] / W2 [HID,D] serve directly as matmul lhsT operands, the SE
pool is a free-dim reduction, the gate is a native per-partition scalar
multiply, and the device never transposes anything.

Tail-overlap strategy: the SE gate depends on the token-mean of out, which
would serialize the entire 8 MB output DMA after the last GEMM2. Instead
GEMM2 for the last two token chunks is deferred until after the gate:
  - their GEMM1 runs spread through the main loop (2 extra heads per
    chunk) so the scalar engine's gelu stream never becomes the pacer,
    with h kept resident in SBUF and row-sums taken on the DVE;
  - pooled = (sum_chunk prow + hsum67 @ W2)/N + b2/4, where prow comes
    free from the pre-gate GEMM2 epilogues' accum_out and the matvec is
    64 N=1 matmuls interleaved into the last pre-gate chunk;
  - after the tiny SE MLP produces the gate, the deferred GEMM2 runs with
    (1+gate) folded into its ACT epilogue while the DVE rescales the six
    retained chunks - so the whole 8 MB output DMA overlaps the final
    ~27us of matmuls instead of trailing them.
"""

import numpy as np
import ml_dtypes

B = 8
N = 4096
DIM = 1024
H = 4
HD = 256           # head dim
HID = 1024         # per-head hidden
SQ = 64            # squeeze dim
TCH = 512          # tokens per chunk
NCHUNK = N // TCH  # 8
PRE = 6            # chunks whose GEMM2 runs before the gate
NCORES = 8

_BF = ml_dtypes.bfloat16

_cache = {}


def _build():
    from contextlib import ExitStack

    import concourse.bass as bass
    import concourse.mybir as mybir
    from concourse import bacc
    from concourse.tile import TileContext

    dt = mybir.dt
    bf = dt.bfloat16
    f32 = dt.float32
    Act = mybir.ActivationFunctionType
    Alu = mybir.AluOpType
    Ax = mybir.AxisListType

    nc = bacc.Bacc("TRN2", target_bir_lowering=False, debug=False)

    xt = nc.dram_tensor("xt", [DIM, N], bf, kind="ExternalInput")
    w1 = nc.dram_tensor("w1", [H, HD, HID], bf, kind="ExternalInput")
    w2 = nc.dram_tensor("w2", [H, HID, HD], bf, kind="ExternalInput")
    b1t = nc.dram_tensor("b1t", [128, H * 8], f32, kind="ExternalInput")
    b2t = nc.dram_tensor("b2t", [128, 8], f32, kind="ExternalInput")
    cw1 = nc.dram_tensor("cw1", [DIM, SQ], bf, kind="ExternalInput")
    cb1t = nc.dram_tensor("cb1t", [SQ, 1], f32, kind="ExternalInput")
    cw2 = nc.dram_tensor("cw2", [SQ, DIM], bf, kind="ExternalInput")
    cb2t = nc.dram_tensor("cb2t", [128, 8], f32, kind="ExternalInput")
    outT = nc.dram_tensor("outT", [DIM, N], bf, kind="ExternalOutput")

    with TileContext(nc) as tc, ExitStack() as ctx:
        const = ctx.enter_context(tc.tile_pool(name="const", bufs=1))
        xpool = ctx.enter_context(tc.tile_pool(name="xpool", bufs=2))
        hpool = ctx.enter_context(tc.tile_pool(name="hpool", bufs=2))
        pg1 = ctx.enter_context(tc.tile_pool(name="pg1", bufs=4, space="PSUM"))
        pg2 = ctx.enter_context(tc.tile_pool(name="pg2", bufs=3, space="PSUM"))
        pgp = ctx.enter_context(tc.tile_pool(name="pgp", bufs=1, space="PSUM"))

        # ---- activation-table + PE-clock warmup (overlaps the load phase) ----
        warm = const.tile([128, 1], f32, name="warm", tag="warm")
        nc.vector.memset(warm, 0.0)
        nc.scalar.activation(out=warm, in_=warm, func=Act.Sigmoid)
        nc.scalar.activation(out=warm, in_=warm, func=Act.Relu)
        nc.scalar.activation(out=warm, in_=warm, func=Act.Gelu)
        # dummy matmuls keep the PE busy through the HAM activity window so
        # the real GEMM stream starts at the warm 2.4 GHz clock
        wmm = const.tile([128, 512], bf, name="wmm", tag="wmm")
        nc.vector.memset(wmm, 0.0)
        for _ in range(6):
            pw = pg1.tile([128, 512], f32, name="p1", tag="p1")
            nc.tensor.matmul(pw, lhsT=wmm[:, 0:128], rhs=wmm,
                             start=True, stop=True)

        # ---- persistent tiles ----
        w1sb = [const.tile([128, 2, HID], bf, name=f"w1sb_{h}",
                           tag=f"w1sb_{h}") for h in range(H)]
        w2sb = [const.tile([128, 8, HD], bf, name=f"w2sb_{h}",
                           tag=f"w2sb_{h}") for h in range(H)]
        b1sb = const.tile([128, H * 8], f32, name="b1sb", tag="b1sb")
        b2sb = const.tile([128, 8], f32, name="b2sb", tag="b2sb")
        # channel-major out accumulator (pre-gate chunks only; the two
        # post-gate chunks stage through the by-then-dead x67 tiles)
        oT = [const.tile([128, PRE * TCH], bf, name=f"oT_{c}",
                         tag=f"oT_{c}") for c in range(8)]
        # x and h kept resident for the post-gate chunks
        x67 = {c: [const.tile([128, TCH], bf, name=f"x{c}_{ct}",
                              tag=f"x{c}_{ct}") for ct in range(8)]
               for c in (6, 7)}
        h67 = {c: [const.tile([128, TCH], bf, name=f"h{c}_{i}",
                              tag=f"h{c}_{i}") for i in range(32)]
               for c in (6, 7)}
        # per-(chunk, head, m) gelu row sums for chunks 6/7 (DVE-reduced)
        hrow = const.tile([128, 64], f32, name="hrow", tag="hrow")
        hsumb = const.tile([128, 32], bf, name="hsumb", tag="hsumb")
        # per-(chunk, ctile) out row sums from the pre-gate GEMM2 epilogues
        prow = const.tile([128, PRE * 8], f32, name="prow", tag="prow")
        prow3 = prow.rearrange("p (i q) -> p i q", q=8)
        prsum = const.tile([128, 8], f32, name="prsum", tag="prsum")
        cw1sb = const.tile([128, 8, SQ], bf, name="cw1sb", tag="cw1sb")
        cb1sb = const.tile([SQ, 1], f32, name="cb1sb", tag="cb1sb")
        cw2sb = const.tile([SQ, DIM], bf, name="cw2sb", tag="cw2sb")
        cb2sb = const.tile([128, 8], f32, name="cb2sb", tag="cb2sb")
        b2q = const.tile([128, 8], f32, name="b2q", tag="b2q")
        # pooled matvec accumulator (held across the interleaved emissions)
        poolp = pgp.tile([128, 8], f32, name="poolp", tag="poolp")

        def prefetch_x(c, eng=None):
            tiles = []
            for ct in range(8):
                if c >= PRE:
                    t = x67[c][ct]
                else:
                    t = xpool.tile([128, TCH], bf, name=f"x_{ct}",
                                   tag=f"x_{ct}")
                (eng or nc.sync).dma_start(
                    out=t, in_=xt[ct * 128:(ct + 1) * 128,
                                  c * TCH:(c + 1) * TCH])
                tiles.append(t)
            return tiles

        # ---- load order: one serial sync HWDGE ring, strict deadline
        # order (x chunk 0 first, then weights interleaved by first use) ----
        nc.sync.dma_start(out=b1sb, in_=b1t[:, :])
        w1r0 = w1[0].rearrange("(k p) n -> p k n", p=128)

        def x0_tile(ct):
            t = xpool.tile([128, TCH], bf, name=f"x_{ct}", tag=f"x_{ct}")
            nc.sync.dma_start(out=t, in_=xt[ct * 128:(ct + 1) * 128, 0:TCH])
            return t

        # head-0 weights split into m-pair slices interleaved with the
        # first x tiles: the first matmul only waits for ~400KB of DMA
        nc.sync.dma_start(out=w1sb[0][:, :, 0:256], in_=w1r0[:, :, 0:256])
        xcur = [None] * 8
        xcur[0] = x0_tile(0)
        xcur[1] = x0_tile(1)
        nc.sync.dma_start(out=b2sb, in_=b2t[:, :])
        for mp in range(1, 4):
            nc.sync.dma_start(out=w1sb[0][:, :, mp * 256:(mp + 1) * 256],
                              in_=w1r0[:, :, mp * 256:(mp + 1) * 256])
        w2r0 = w2[0].rearrange("(k p) n -> p k n", p=128)
        nc.sync.dma_start(out=w2sb[0][:, 0:4, :], in_=w2r0[:, 0:4, :])
        nc.sync.dma_start(out=w2sb[0][:, 4:8, :], in_=w2r0[:, 4:8, :])
        xcur[2] = x0_tile(2)
        xcur[3] = x0_tile(3)
        nc.sync.dma_start(out=w1sb[1],
                          in_=w1[1].rearrange("(k p) n -> p k n", p=128))
        xcur[4] = x0_tile(4)
        xcur[5] = x0_tile(5)
        nc.sync.dma_start(out=w2sb[1],
                          in_=w2[1].rearrange("(k p) n -> p k n", p=128))
        xcur[6] = x0_tile(6)
        xcur[7] = x0_tile(7)
        for h in range(2, H):
            nc.sync.dma_start(out=w1sb[h],
                              in_=w1[h].rearrange("(k p) n -> p k n", p=128))
            nc.sync.dma_start(out=w2sb[h],
                              in_=w2[h].rearrange("(k p) n -> p k n", p=128))
        xnext = prefetch_x(1)
        prefetch_x(6)
        nc.sync.dma_start(out=cb1sb, in_=cb1t[:, :])
        nc.sync.dma_start(out=cb2sb, in_=cb2t[:, :])
        nc.sync.dma_start(out=cw1sb,
                          in_=cw1.rearrange("(c p) n -> p c n", p=128))
        nc.sync.dma_start(out=cw2sb, in_=cw2[:, :])
        nc.vector.tensor_scalar_mul(b2q, b2sb, (NCHUNK - PRE) * TCH / N)

        def emit_g1(c, h, xts, dest):
            # GEMM1 head: dest[m] = gelu(W1_h^T x^T + b1) for 8 m-tiles
            for m in range(8):
                p1 = pg1.tile([128, TCH], f32, name="p1", tag="p1")
                nc.tensor.matmul(
                    p1, lhsT=w1sb[h][:, 0, m * 128:(m + 1) * 128],
                    rhs=xts[2 * h], start=True, stop=False)
                nc.tensor.matmul(
                    p1, lhsT=w1sb[h][:, 1, m * 128:(m + 1) * 128],
                    rhs=xts[2 * h + 1], start=False, stop=True)
                nc.scalar.activation(
                    out=dest[m], in_=p1, func=Act.Gelu,
                    bias=b1sb[:, h * 8 + m:h * 8 + m + 1])

        def emit_matvec(h):
            # pooled contribution of chunks 6/7: poolp[:, 2h+d] = sum_k
            # W2_h[k-block, d-half]^T hsum67_h[k-block]  (64 N=1 matmuls)
            for d in range(2):
                col = h * 2 + d
                for k in range(8):
                    nc.tensor.matmul(
                        poolp[:, col:col + 1],
                        lhsT=w2sb[h][:, k, d * 128:(d + 1) * 128],
                        rhs=hsumb[:, h * 8 + k:h * 8 + k + 1],
                        start=(k == 0), stop=(k == 7))

        # deferred-GEMM1 slots: 2 per chunk at c=1..4 covers chunks 6,7
        slots = {1: [(6, 0), (6, 1)], 2: [(6, 2), (6, 3)],
                 3: [(7, 0), (7, 1)], 4: [(7, 2), (7, 3)]}

        def emit_mix(cur, prev):
            # cur:  None | ("pre", c, h) | ("slot", dc, dh)  - a GEMM1 unit
            # prev: None | (c, h, ht)                        - a GEMM2 unit
            # Interleaving the GEMM1 matmul pairs with the previous unit's
            # GEMM2 k-chain halves the gelu-tile production rate so the
            # scalar engine (~590ns/tile) never stalls the PE.
            ht = None
            if cur is not None:
                if cur[0] == "pre":
                    _, c, h = cur
                    xts = xcur
                    dest = [hpool.tile([128, TCH], bf, name=f"ht_{m}",
                                       tag=f"ht_{m}") for m in range(8)]
                    ht = dest
                else:
                    _, dc, dh = cur
                    h = dh
                    xts = x67[dc]
                    dest = h67[dc][dh * 8:(dh + 1) * 8]
            if prev is not None:
                pc, ph, pht = prev
                p2 = [pg2.tile([128, TCH], f32, name="p2", tag="p2")
                      for _ in range(2)]
            for m in range(8):
                if cur is not None:
                    p1 = pg1.tile([128, TCH], f32, name="p1", tag="p1")
                    nc.tensor.matmul(
                        p1, lhsT=w1sb[h][:, 0, m * 128:(m + 1) * 128],
                        rhs=xts[2 * h], start=True, stop=False)
                    nc.tensor.matmul(
                        p1, lhsT=w1sb[h][:, 1, m * 128:(m + 1) * 128],
                        rhs=xts[2 * h + 1], start=False, stop=True)
                    nc.scalar.activation(
                        out=dest[m], in_=p1, func=Act.Gelu,
                        bias=b1sb[:, h * 8 + m:h * 8 + m + 1])
                if prev is not None:
                    for d in range(2):
                        nc.tensor.matmul(
                            p2[d],
                            lhsT=w2sb[ph][:, m, d * 128:(d + 1) * 128],
                            rhs=pht[m], start=(m == 0), stop=(m == 7))
            if prev is not None:
                for d in range(2):
                    ctile = ph * 2 + d
                    idx = pc * 8 + ctile
                    nc.vector.tensor_scalar(
                        out=oT[ctile][:, pc * TCH:(pc + 1) * TCH],
                        in0=p2[d], scalar1=b2sb[:, ctile:ctile + 1],
                        scalar2=0.0, op0=Alu.add, op1=Alu.add,
                        accum_out=prow[:, idx:idx + 1])
            if cur is not None and cur[0] == "slot":
                for m in range(8):
                    col = (dc - 6) * 32 + dh * 8 + m
                    nc.vector.tensor_reduce(
                        out=hrow[:, col:col + 1], in_=dest[m],
                        axis=Ax.X, op=Alu.add)
            return ht

        # ---- main loop: GEMM1 units in order, each interleaved with the
        # oldest pending GEMM2 unit ----
        g2q = []
        for c in range(PRE):
            if 1 <= c < PRE - 1:
                xnext = prefetch_x(c + 1)
            if c == 2:
                prefetch_x(7)
            sl = slots.get(c, ())
            units = [("pre", c, 0), ("pre", c, 1)]
            if sl:
                units.append(("slot",) + sl[0])
            units += [("pre", c, 2), ("pre", c, 3)]
            if sl:
                units.append(("slot",) + sl[1])
            for u in units:
                prev = g2q.pop(0) if g2q else None
                ht = emit_mix(u, prev)
                if u[0] == "pre":
                    g2q.append((u[1], u[2], ht))
            if c == PRE - 2:
                # chunks 6+7 h sums, bf16, ready before the matvec emissions
                for h in range(H):
                    nc.vector.tensor_tensor(
                        out=hsumb[:, h * 8:(h + 1) * 8],
                        in0=hrow[:, h * 8:h * 8 + 8],
                        in1=hrow[:, 32 + h * 8:32 + h * 8 + 8], op=Alu.add)
            xcur = xnext

        # drain: the last pending GEMM2 interleaved with the pooled matvec
        assert len(g2q) == 1
        pc, ph, pht = g2q[0]
        p2 = [pg2.tile([128, TCH], f32, name="p2", tag="p2")
              for _ in range(2)]
        mvlist = [(h, d, k) for h in range(H) for d in range(2)
                  for k in range(8)]
        for m in range(8):
            for d in range(2):
                nc.tensor.matmul(
                    p2[d], lhsT=w2sb[ph][:, m, d * 128:(d + 1) * 128],
                    rhs=pht[m], start=(m == 0), stop=(m == 7))
            for (h, d, k) in mvlist[m * 8:(m + 1) * 8]:
                nc.tensor.matmul(
                    poolp[:, h * 2 + d:h * 2 + d + 1],
                    lhsT=w2sb[h][:, k, d * 128:(d + 1) * 128],
                    rhs=hsumb[:, h * 8 + k:h * 8 + k + 1],
                    start=(k == 0), stop=(k == 7))
        for d in range(2):
            ctile = ph * 2 + d
            idx = pc * 8 + ctile
            nc.vector.tensor_scalar(
                out=oT[ctile][:, pc * TCH:(pc + 1) * TCH], in0=p2[d],
                scalar1=b2sb[:, ctile:ctile + 1],
                scalar2=0.0, op0=Alu.add, op1=Alu.add,
                accum_out=prow[:, idx:idx + 1])

        # prefetch the sigmoid table set (the gelu set is resident; without
        # this the 1.3us table load lands between relu and the gate sigmoid)
        nc.scalar.activation(out=warm, in_=warm, func=Act.Sigmoid)

        # ---- SE channel attention:
        # pooled = (sum_c prow_c + poolp)/N + b2/4 ----
        for q in range(8):
            nc.vector.tensor_reduce(out=prsum[:, q:q + 1],
                                    in_=prow3[:, 0:PRE, q],
                                    axis=Ax.X, op=Alu.add)
        pool1 = const.tile([128, 8], f32, name="pool1", tag="pool1")
        nc.vector.tensor_tensor(out=pool1, in0=prsum, in1=poolp, op=Alu.add)
        pooltmp = const.tile([128, 8], f32, name="pooltmp", tag="pooltmp")
        nc.vector.tensor_scalar_mul(pooltmp, pool1, 1.0 / N)
        pooledT = const.tile([128, 8], bf, name="pooledT", tag="pooledT")
        nc.vector.tensor_tensor(out=pooledT, in0=pooltmp, in1=b2q,
                                op=Alu.add)

        pz = pg1.tile([SQ, 1], f32, name="pz", tag="p1")
        for cb in range(8):
            nc.tensor.matmul(pz, lhsT=cw1sb[:, cb, :],
                             rhs=pooledT[:, cb:cb + 1],
                             start=(cb == 0), stop=(cb == 7))
        z1sb = const.tile([SQ, 1], bf, name="z1sb", tag="z1sb")
        nc.scalar.activation(out=z1sb, in_=pz, func=Act.Relu, bias=cb1sb)

        gp8 = pgp.tile([128, 8], f32, name="gp8", tag="poolp")
        for cb in range(8):
            nc.tensor.matmul(gp8[:, cb:cb + 1],
                             lhsT=cw2sb[:, cb * 128:(cb + 1) * 128],
                             rhs=z1sb, start=True, stop=True)
        gadd = const.tile([128, 8], f32, name="gadd", tag="gadd")
        nc.vector.tensor_tensor(out=gadd, in0=gp8, in1=cb2sb, op=Alu.add)
        g1T = const.tile([128, 8], f32, name="g1T", tag="g1T")
        nc.scalar.activation(out=g1T, in_=gadd, func=Act.Sigmoid)
        nc.vector.tensor_scalar_add(g1T, g1T, 1.0)
        # gated bias: b2*(1+g), so (p2 + b2)*(1+g) = p2*g1 + b2g
        b2g = const.tile([128, 8], f32, name="b2g", tag="b2g")
        nc.vector.tensor_tensor(out=b2g, in0=b2sb, in1=g1T, op=Alu.mult)

        # ---- post-gate: GEMM2 for chunks 6,7 (gate folded into the ACT
        # epilogue) while the DVE rescales chunks 0..5 in place; every piece
        # is DMA'd out as soon as it is gated, overlapping the matmuls ----
        PW = (PRE * TCH) // 2  # in-place scale piece width
        for pct in range(8):
            for ph in range(2):
                psl = slice(ph * PW, (ph + 1) * PW)
                nc.vector.tensor_scalar(
                    out=oT[pct][:, psl], in0=oT[pct][:, psl],
                    scalar1=g1T[:, pct:pct + 1], scalar2=0.0,
                    op0=Alu.mult, op1=Alu.add)
                nc.sync.dma_start(
                    out=outT[pct * 128:(pct + 1) * 128, psl],
                    in_=oT[pct][:, psl])
        for c in (6, 7):
            for h in range(H):
                for d in range(2):
                    ctile = h * 2 + d
                    p2 = pg2.tile([128, TCH], f32, name="p2", tag="p2")
                    for k in range(8):
                        nc.tensor.matmul(
                            p2, lhsT=w2sb[h][:, k, d * 128:(d + 1) * 128],
                            rhs=h67[c][h * 8 + k],
                            start=(k == 0), stop=(k == 7))
                    st = x67[c][ctile]
                    nc.scalar.activation(
                        out=st, in_=p2, func=Act.Identity,
                        scale=g1T[:, ctile:ctile + 1],
                        bias=b2g[:, ctile:ctile + 1])
                    nc.scalar.dma_start(
                        out=outT[ctile * 128:(ctile + 1) * 128,
                                 c * TCH:(c + 1) * TCH],
                        in_=st)

    nc.compile()
    return nc


def _get_nc():
    if "nc" not in _cache:
        _cache["nc"] = _build()
    return _cache["nc"]


def _make_in_maps(x, W1, b1, W2, b2, cw1, cb1, cw2, cb2):
    # bf16 + pre-transposed x: (B, N, DIM) -> per-core (DIM, N)
    xb = np.asarray(x, dtype=_BF)
    w1b = np.asarray(W1, dtype=_BF)
    w2b = np.asarray(W2, dtype=_BF)
    cw1b = np.asarray(cw1, dtype=_BF)
    cw2b = np.asarray(cw2, dtype=_BF)
    b1tv = np.ascontiguousarray(
        np.asarray(b1, np.float32).reshape(H, 8, 128).transpose(2, 0, 1)
        .reshape(128, H * 8))
    b2tv = np.ascontiguousarray(
        np.asarray(b2, np.float32).reshape(H, 2, 128).transpose(2, 0, 1)
        .reshape(128, 8))
    cb1v = np.asarray(cb1, np.float32).reshape(SQ, 1)
    cb2tv = np.ascontiguousarray(
        np.asarray(cb2, np.float32).reshape(8, 128).T)

    shared = {
        "w1": w1b, "w2": w2b, "b1t": b1tv, "b2t": b2tv,
        "cw1": cw1b, "cb1t": cb1v, "cw2": cw2b, "cb2t": cb2tv,
    }
    return [dict(shared, xt=np.ascontiguousarray(xb[i].T))
            for i in range(NCORES)]


def kernel(x, W1, b1, W2, b2, cw1, cb1, cw2, cb2):
    from concourse.bass_utils import run_bass_kernel_spmd

    nc = _get_nc()
    in_maps = _make_in_maps(x, W1, b1, W2, b2, cw1, cb1, cw2, cb2)
    res = run_bass_kernel_spmd(nc, in_maps, core_ids=list(range(NCORES)))
    # un-transpose: per-core (DIM, N) -> (N, DIM)
    y = np.stack([res.results[i]["outT"].T for i in range(NCORES)], axis=0)
    return y.astype(np.float32)
